# revision 31
# baseline (speedup 1.0000x reference)
"""Trainium2 Bass kernel for BioBERT-ARG-GNN (gated pooling + 2-layer GCN + MLP head).

Strategy: pure data parallel over batch B=64 across 8 NeuronCores (8 graphs
per core).  All index-derived structure is precomputed on the host from the
int tensors (submap / edge_index):

  - P'[s,n]  : subtoken->node one-hot with 1/max(cnt,1) folded in (bf16)
  - Ahat[s,d]: dense normalized adjacency D^-1/2 (A+I) D^-1/2 (bf16)

lh is host-cast to bf16 (halves HBM traffic, 3.5x faster matmuls vs fp32
HIGH mode).  The dataflow is fully transpose-free by keeping pooled
features transposed (NFT[h,n] = sum_s lh[s,h]*g[s]*P'[s,n]) and
alternating matmul operand roles through the GCN:

  NFT -> T1[n,gh] -> Z1T[gh,d] -> T2[n,gh2] -> Z2[d,gh2] -> pooled (PE matvec)

Engine balance for the per-token gate sigmoid(lh.wr+br): one DVE 2x-mode
product over chunks 0-2 (+ GpSimd product for chunk 3), free-axis
accumulates split DVE(c0,c3)/ACT(c1,c2); sigmoid + big PSUM copies on ACT;
P'*gate on GpSimd; GCN relu/copy moves on DVE.  Token layout s = 4p + c
gives contiguous per-partition DMA; gd is split into two transfers so the
gate product starts after the first 60% of a graph's data lands.
"""

import os
import sys

import numpy as np

for _p in ("/opt/trn_rl_repo", "/root/.axon_site/_ro/trn_rl_repo"):
    if os.path.isdir(_p) and _p not in sys.path:
        sys.path.insert(0, _p)

import ml_dtypes  # noqa: E402
import concourse.bass as bass  # noqa: E402
import concourse.mybir as mybir  # noqa: E402
from concourse import tile  # noqa: E402
from concourse.bass_utils import run_bass_kernel_spmd  # noqa: E402

# Problem shapes (hardcoded per contest rules).
B, S, H = 64, 512, 768
N, E = 128, 1024
GH, FH, L = 128, 256, 2
NCORES = 8
BL = B // NCORES   # graphs per core
SC = S // 128      # token chunks per graph (s = 4p + c)
HC = H // 128      # BERT-hidden 128-chunks
FC = (H + GH) // 128  # concat-feature chunks for the FC head

# gd packing (per graph, per partition, bf16 elements); DMA'd as two
# pieces: [0 : 3H) and [3H : GDW)
OLH = 0            # [SC*H]  lh, token 4p+c at [c*H : (c+1)*H]
OPP = SC * H       # [SC*N]  P' one-hot * invc
OAT = OPP + SC * N  # [N]    Ahat[src=p, dst]
GDW = OAT + N
GSPLIT = 4 * H

# cw packing (weights/constants, per partition, bf16 elements); wr ships
# separately (cwh) so the gate product isn't blocked behind the weights.
OW1 = 0            # [HC*GH] W1 tiled (hp, t*GH+gh) = W1[t*128+hp, gh]
OW2 = OW1 + HC * GH   # [GH] W2
OWF1 = OW2 + GH    # [FC*FH] Wf1 tiled, pooled block pre-divided by N
OCLS = OWF1 + FC * FH  # [HC*BL] cls^T
OMC = OCLS + HC * BL   # [1]   1/N mean column
CWW = OMC + 1

f32 = mybir.dt.float32
bf16 = mybir.dt.bfloat16
AFT = mybir.ActivationFunctionType
ALU = mybir.AluOpType
BF16 = ml_dtypes.bfloat16

_CACHE = {}


def _split_multi_waits(nc: bass.Bass) -> int:
    """Walrus in this container accepts one sync-wait per instruction; split
    extra waits into single-wait EventSemaphore nops just before it."""
    n_split = 0
    for fn in nc.m.functions:
        for blk in fn.blocks:
            new_instrs = []
            changed = False
            for inst in blk.instructions:
                si = getattr(inst, "sync_info", None)
                if si is not None and si.on_wait is not None and len(si.on_wait) > 1:
                    waits = list(si.on_wait)
                    for j, w in enumerate(waits[:-1]):
                        ev = mybir.InstEventSemaphore(
                            name=f"{inst.name}_ws{j}",
                            ins=[], outs=[],
                            engine=inst.engine,
                            sync_info=mybir.SyncInfo(on_wait=[w], on_update=[]),
                        )
                        new_instrs.append(ev)
                    inst.sync_info = mybir.SyncInfo(
                        on_wait=[waits[-1]], on_update=list(si.on_update))
                    n_split += 1
                    changed = True
                new_instrs.append(inst)
            if changed:
                blk.instructions = new_instrs
    return n_split


def build_program(br_val: float, b1_zero: bool, b2_zero: bool,
                  bf1_zero: bool, bf2_zero: bool) -> bass.Bass:
    nc = bass.Bass()

    gd_d = nc.declare_dram_parameter("gd", [BL, 128, GDW], bf16, isOutput=False)
    cwh_d = nc.declare_dram_parameter("cwh", [128, H], bf16, isOutput=False)
    cw_d = nc.declare_dram_parameter("cw", [128, CWW], bf16, isOutput=False)
    wf2r_d = nc.declare_dram_parameter("wf2r", [BL, L * FH], f32, isOutput=False)
    b1c_d = nc.declare_dram_parameter("b1c", [128, 1], f32, isOutput=False)
    b2c_d = nc.declare_dram_parameter("b2c", [128, 1], f32, isOutput=False)
    bf1r_d = nc.declare_dram_parameter("bf1r", [BL, FH], f32, isOutput=False)
    bf2r_d = nc.declare_dram_parameter("bf2r", [BL, L], f32, isOutput=False)
    out_d = nc.declare_dram_parameter("out", [BL, L], f32, isOutput=True)

    with tile.TileContext(nc) as tc:
        with (
            tc.tile_pool(name="const", bufs=1) as cpool,
            tc.tile_pool(name="gdp", bufs=BL) as gdpool,
            tc.tile_pool(name="work", bufs=3) as wpool,
            tc.tile_pool(name="small", bufs=4) as spool,
            tc.tile_pool(name="psA", bufs=2, space="PSUM") as psA,
            tc.tile_pool(name="psB", bufs=2, space="PSUM") as psB,
            tc.tile_pool(name="psC", bufs=1, space="PSUM") as psC,
        ):
            # ---- DMAs (HWDGE on SP ring): wr first, then graph data ----
            cwh = cpool.tile([128, H], bf16)
            nc.sync.dma_start(cwh[:], cwh_d[:])
            gdas, gdbs = [], []
            for g in range(BL):
                gda = gdpool.tile([128, GSPLIT], bf16, tag="gda", bufs=BL)
                nc.sync.dma_start(gda[:], gd_d[g][:, 0:GSPLIT])
                gdb = gdpool.tile([128, GDW - GSPLIT], bf16, tag="gdb",
                                  bufs=BL)
                nc.sync.dma_start(gdb[:], gd_d[g][:, GSPLIT:GDW])
                gdas.append(gda)
                gdbs.append(gdb)
            cw = cpool.tile([128, CWW], bf16)
            nc.sync.dma_start(cw[:], cw_d[:])
            wf2rs = cpool.tile([BL, L * FH], f32)
            nc.sync.dma_start(wf2rs[:], wf2r_d[:])
            b1cs = b2cs = bf1rs = bf2rs = None
            if not b1_zero:
                b1cs = cpool.tile([128, 1], f32)
                nc.sync.dma_start(b1cs[:], b1c_d[:])
            if not b2_zero:
                b2cs = cpool.tile([128, 1], f32)
                nc.sync.dma_start(b2cs[:], b2c_d[:])
            if not bf1_zero:
                bf1rs = cpool.tile([BL, FH], f32)
                nc.sync.dma_start(bf1rs[:], bf1r_d[:])
            if not bf2_zero:
                bf2rs = cpool.tile([BL, L], f32)
                nc.sync.dma_start(bf2rs[:], bf2r_d[:])

            # persistent scratch / accumulators
            scr2d = cpool.tile([128, H], bf16)   # DVE reduce throwaway out
            scr2a = cpool.tile([128, H], bf16)   # ACT reduce throwaway out
            x2scr = cpool.tile([128, GH], bf16)  # (b2 nonzero path)
            poolsb = cpool.tile([128, BL], f32) if not b2_zero else None
            wrv = cwh[:]
            wr3 = cwh[:].rearrange("p (o h) -> p o h", o=1).broadcast_to(
                [128, SC, H])
            mcol = cw[:, OMC:OMC + 1]

            # psC: pooled columns and the FC1 psum in separate banks so
            # the cls-chunk matmuls can run ahead of the graph loop
            poolt = psC.tile([128, BL], f32, tag="pool")
            h1t = psC.tile([BL, FH], f32, tag="h1")
            pool_ps = poolt[:]
            h1_ps = h1t[:]

            # FC head cls chunks only need cw; the pooled chunk closes the
            # accumulation group after the loop.
            for c in range(HC):
                nc.tensor.matmul(h1_ps, cw[:, OCLS + c * BL:OCLS + (c + 1) * BL],
                                 cw[:, OWF1 + c * FH:OWF1 + (c + 1) * FH],
                                 start=(c == 0), stop=False)

            nfts_l = [None] * BL
            ativ_l = [None] * BL

            def phase1(g):
                gda, gdb = gdas[g], gdbs[g]
                ativ_l[g] = gdb[:, OAT - GSPLIT:OAT - GSPLIT + N]

                # --- gate logits: DVE products (chunks 0-2, then 3);
                # accumulates split DVE(c0,c3) / ACT(c1,c2)
                logits4 = spool.tile([128, SC], f32, tag="lg", bufs=8)
                scr = wpool.tile([128, SC, H], bf16, tag="scr", bufs=3)
                nc.vector.tensor_tensor(
                    out=scr[:], in0=gda[:].rearrange("p (c h) -> p c h", c=SC),
                    in1=wr3, op=ALU.mult)
                nc.vector.tensor_scalar(
                    scr2d[:], scr[:, 0, :], 0.0, None, ALU.bypass,
                    ALU.add, accum_out=logits4[:, 0:1])
                for c in (1, 2):
                    nc.scalar.activation(scr2a[:], scr[:, c, :], AFT.Copy,
                                         accum_out=logits4[:, c:c + 1])
                if g % 2 == 0:
                    nc.vector.tensor_scalar(
                        scr2d[:], scr[:, 3, :], 0.0, None, ALU.bypass,
                        ALU.add, accum_out=logits4[:, 3:4])
                else:
                    nc.scalar.activation(scr2a[:], scr[:, 3, :], AFT.Copy,
                                         accum_out=logits4[:, 3:4])
                gate4 = spool.tile([128, SC], f32, tag="gt", bufs=8)
                if br_val == 0.0:
                    nc.scalar.activation(gate4[:], logits4[:], AFT.Sigmoid)
                else:
                    nc.scalar.activation(gate4[:], logits4[:], AFT.Sigmoid,
                                         bias=float(br_val))

                # --- Pg = P' * gate: chunks 0-1 on DVE (fast, unblocks the
                # NFT matmuls at once); chunks 2-3 on GpSimd (off the
                # critical path by the time the PE reaches them)
                pg = wpool.tile([128, SC, N], bf16, tag="pg", bufs=3)
                ppv = gdb[:, OPP - GSPLIT:OPP - GSPLIT + SC * N]
                nc.gpsimd.tensor_tensor(
                    out=pg[:],
                    in0=ppv[:].rearrange("p (c n) -> p c n", c=SC),
                    in1=gate4[:].broadcast_to([128, SC, N]), op=ALU.mult)

                # --- pooled features, transposed: NFT[h,n] += lh_c,t^T @ Pg_c
                nft_ps = psA.tile([128, HC, GH], f32, tag="nft", bufs=2)
                for c in range(SC):
                    for t in range(HC):
                        nc.tensor.matmul(
                            nft_ps[:, t, :],
                            gda[:, c * H + t * 128:c * H + (t + 1) * 128],
                            pg[:, c, :],
                            start=(c == 0), stop=(c == SC - 1))
                nfts_l[g] = nft_ps

            def phase2(g):
                nfts = wpool.tile([128, HC, GH], bf16, tag="nfts", bufs=3)
                nc.scalar.activation(nfts[:], nfts_l[g][:], AFT.Copy)
                ativ = ativ_l[g]
                # --- GCN chain in one PSUM bank: t1 | z1 | t2 | z2 slices
                gcn_ps = psB.tile([128, 3, GH], f32, tag="mm", bufs=2)
                t1_ps, z1_ps = gcn_ps[:, 0, :], gcn_ps[:, 1, :]
                t2_ps, z2_ps = gcn_ps[:, 2, :], gcn_ps[:, 0, :]
                # layer 1: T1[n,gh] = NF @ W1 ; Z1T[gh,d] = T1^T Ahat
                for t in range(HC):
                    nc.tensor.matmul(
                        t1_ps, nfts[:, t, :],
                        cw[:, OW1 + t * GH:OW1 + (t + 1) * GH],
                        start=(t == 0), stop=(t == HC - 1))
                t1s = spool.tile([128, GH], bf16, tag="t1", bufs=12)
                nc.scalar.activation(t1s[:], t1_ps, AFT.Copy)
                nc.tensor.matmul(z1_ps, t1s[:], ativ, start=True, stop=True)
                x1ts = spool.tile([128, GH], bf16, tag="t1", bufs=12)
                if b1_zero:
                    nc.vector.tensor_scalar_max(x1ts[:], z1_ps, 0.0)
                else:
                    nc.scalar.activation(x1ts[:], z1_ps, AFT.Relu,
                                         bias=b1cs[:])

                # layer 2
                nc.tensor.matmul(t2_ps, x1ts[:],
                                 cw[:, OW2:OW2 + GH], start=True, stop=True)
                t2s = spool.tile([128, GH], bf16, tag="t1", bufs=12)
                nc.vector.tensor_copy(t2s[:], t2_ps)
                if b2_zero:
                    # Z2[d,gh2]; relu on DVE; graph-mean via PE matvec column
                    nc.tensor.matmul(z2_ps, ativ, t2s[:], start=True, stop=True)
                    x2s = spool.tile([128, GH], bf16, tag="t1", bufs=12)
                    nc.vector.tensor_scalar_max(x2s[:], z2_ps, 0.0)
                    nc.tensor.matmul(pool_ps[:, g:g + 1], x2s[:], mcol,
                                     start=True, stop=True)
                else:
                    # Z2T[gh2,d]; relu+bias+free-axis pool accum on ACT
                    nc.tensor.matmul(z2_ps, t2s[:], ativ, start=True, stop=True)
                    nc.scalar.activation(x2scr[:], z2_ps, AFT.Relu,
                                         bias=b2cs[:],
                                         accum_out=poolsb[:, g:g + 1])

            phase1(0)
            for g in range(1, BL):
                phase1(g)
                phase2(g - 1)
            phase2(BL - 1)

            # ---------- FC head over all BL graphs ----------
            pooledb = cpool.tile([128, BL], bf16)
            if b2_zero:
                nc.scalar.activation(pooledb[:], pool_ps, AFT.Copy)
            else:
                nc.scalar.activation(pooledb[:], poolsb[:], AFT.Copy)
            nc.tensor.matmul(h1_ps, pooledb[:],
                             cw[:, OWF1 + HC * FH:OWF1 + FC * FH],
                             start=False, stop=True)
            hr = spool.tile([BL, FH], f32, tag="hr")
            if bf1_zero:
                nc.scalar.activation(hr[:], h1_ps, AFT.Relu)
            else:
                h1t = spool.tile([BL, FH], f32, tag="hr")
                nc.vector.tensor_tensor(out=h1t[:], in0=h1_ps,
                                        in1=bf1rs[:], op=ALU.add)
                nc.vector.tensor_scalar_max(hr[:], h1t[:], 0.0)
            outs = spool.tile([BL, L], f32, tag="out")
            scrF = spool.tile([BL, FH], f32, tag="hr")
            for l in range(L):
                nc.vector.scalar_tensor_tensor(
                    scrF[:], hr[:], 0.0, wf2rs[:, l * FH:(l + 1) * FH],
                    ALU.bypass, ALU.mult, accum_out=outs[:, l:l + 1])
            if not bf2_zero:
                outs2 = spool.tile([BL, L], f32, tag="out")
                nc.vector.tensor_tensor(out=outs2[:], in0=outs[:],
                                        in1=bf2rs[:], op=ALU.add)
                outs = outs2
            nc.sync.dma_start(out_d[:], outs[:])

    _split_multi_waits(nc)
    return nc


def _prepare_in_maps(inputs):
    lh = np.ascontiguousarray(np.asarray(inputs["last_hidden"], dtype=np.float32))
    submap = np.asarray(inputs["submap"]).astype(np.int64)
    edge = np.asarray(inputs["edge_index"]).astype(np.int64)
    assert lh.shape == (B, S, H)
    assert int(np.asarray(inputs.get("num_nodes", N))) == N

    wr = np.asarray(inputs["wr"], dtype=np.float32)
    br = float(np.asarray(inputs["br"], dtype=np.float32))
    W1 = np.asarray(inputs["W1"], dtype=np.float32)
    b1 = np.asarray(inputs["b1"], dtype=np.float32)
    W2 = np.asarray(inputs["W2"], dtype=np.float32)
    b2 = np.asarray(inputs["b2"], dtype=np.float32)
    Wf1 = np.asarray(inputs["Wf1"], dtype=np.float32)
    bf1 = np.asarray(inputs["bf1"], dtype=np.float32)
    Wf2 = np.asarray(inputs["Wf2"], dtype=np.float32)
    bf2 = np.asarray(inputs["bf2"], dtype=np.float32)

    # ---- host precompute of index-derived structure ----
    cnt = np.zeros((B, N), np.float32)
    np.add.at(cnt, (np.repeat(np.arange(B), S), submap.ravel()), 1.0)
    invc = 1.0 / np.maximum(cnt, 1.0)
    A = np.zeros((B, N, N), np.float32)
    np.add.at(A, (np.repeat(np.arange(B), E),
                  edge[:, 0, :].ravel(), edge[:, 1, :].ravel()), 1.0)
    A += np.eye(N, dtype=np.float32)[None]
    deg = A.sum(axis=1)           # in-degree incl self-loop (>= 1)
    dinv = 1.0 / np.sqrt(deg)
    ahat = A * dinv[:, :, None] * dinv[:, None, :]   # [B, src, dst]

    # token layout s = 4p + c
    lhr = lh.reshape(B, 128, SC * H)
    ppm = ((submap.reshape(B, 128, SC)[..., None] == np.arange(N))
           .astype(np.float32) * invc[:, None, None, :])
    gd = np.concatenate(
        [lhr, ppm.reshape(B, 128, SC * N), ahat], axis=2).astype(BF16)
    assert gd.shape == (B, 128, GDW)

    w1t = W1.reshape(HC, 128, GH).transpose(1, 0, 2).reshape(128, HC * GH)
    wf1m = Wf1.copy()
    wf1m[H:, :] /= N              # fold graph-mean 1/N into pooled block
    wf1t = wf1m.reshape(FC, 128, FH).transpose(1, 0, 2).reshape(128, FC * FH)
    cwh = np.ascontiguousarray(np.broadcast_to(wr, (128, H))).astype(BF16)
    mc = np.full((128, 1), 1.0 / N, np.float32)
    cw_base = np.concatenate([w1t, W2, wf1t], axis=1)

    wf2r = np.ascontiguousarray(
        np.broadcast_to(Wf2.T.reshape(1, L * FH), (BL, L * FH)))
    b1c = np.ascontiguousarray(b1[:, None])
    b2c = np.ascontiguousarray(b2[:, None])
    bf1r = np.ascontiguousarray(np.broadcast_to(bf1, (BL, FH)))
    bf2r = np.ascontiguousarray(np.broadcast_to(bf2, (BL, L)))

    in_maps = []
    for i in range(NCORES):
        sl = slice(i * BL, (i + 1) * BL)
        cls_core = lh[sl][:, 0, :]                      # [BL, H]
        clst = (cls_core.T.reshape(HC, 128, BL)
                .transpose(1, 0, 2).reshape(128, HC * BL))
        cwm = np.concatenate([cw_base, clst, mc], axis=1).astype(BF16)
        assert cwm.shape == (128, CWW)
        in_maps.append({
            "gd": np.ascontiguousarray(gd[sl]),
            "cwh": cwh,
            "cw": np.ascontiguousarray(cwm),
            "wf2r": wf2r,
            "b1c": b1c, "b2c": b2c, "bf1r": bf1r, "bf2r": bf2r,
        })
    flags = (br, bool(np.all(b1 == 0)), bool(np.all(b2 == 0)),
             bool(np.all(bf1 == 0)), bool(np.all(bf2 == 0)))
    return in_maps, flags


def _run(inputs, trace=False):
    in_maps, flags = _prepare_in_maps(inputs)
    key = ("prog",) + flags
    if key not in _CACHE:
        _CACHE[key] = build_program(*flags)
    nc = _CACHE[key]
    res = run_bass_kernel_spmd(nc, in_maps, list(range(NCORES)), trace=trace)
    out = np.concatenate([np.asarray(res.results[i]["out"]) for i in range(NCORES)],
                         axis=0).astype(np.float32)
    return out, res


def kernel(**inputs) -> np.ndarray:
    out, _ = _run(inputs, trace=False)
    return out
